# revision 1
# baseline (speedup 1.0000x reference)
"""GNN message-passing kernel for Trainium2 (8 NeuronCores, SPMD).

Reference computation:
    msg  = x[src] * edge_weight[:, None]
    agg  = segment_sum(msg, dst, N) / max(segment_sum(1, dst, N), 1)
    out  = x + alpha * (agg @ W.T + b)

Sharding: nodes are sharded across 8 cores by contiguous ranges; edges are
partitioned by dst so scatter-adds stay local; x is replicated to every
core's DRAM so the src-row gather is always local.

Per core, dst nodes are grouped into 128-node blocks. A block's incoming
edges are processed in chunks of 128 edges: a dma_gather (InstDMAGatherAnt)
fetches the 128 src rows (one per partition), a single DVE tensor_scalar
builds the weighted one-hot selection matrix Sw[e, j] = w[e]*(slot[e]==j),
and the tensor engine accumulates aggT[f, n] += Xg[e, f]^T @ Sw[e, n] in
PSUM. A second matmul applies W^T, the scalar engine scales by alpha/deg,
and DVE adds bias and residual.

dma_gather uses int16 indices, so src space is split into 4 buckets of
<= 32768 rows; each chunk's edges come from a single (block, bucket) group.
Gathers are batched (one dma_gather per bucket per ~12-block batch) to
amortize the ~1us SWDGE per-instruction overhead.

All 8 cores run one shared program: each core orders its blocks by
descending chunk count and the program uses the per-position max, so the
control flow is identical and only the data differs.
"""

import numpy as np

P = 128
NCORES = 8
NBUCK = 4

# set by test harness for profiling; grading leaves these defaults
TRACE = False
LAST_RESULTS = None
GATHER_BLOCKS = 12      # block-slots per gather batch
MAX_GATHER_CHUNKS = 8   # chunks (x128 idx) per dma_gather call (SWDGE ring cap)


def _preprocess(x, src, dst, w):
    N, D = x.shape
    E = src.shape[0]
    SH = -(-N // NCORES)          # nodes per core shard
    NBLK = -(-SH // P)            # 128-node blocks per core
    SHP = NBLK * P                # padded shard size
    BSZ = -(-N // NBUCK)          # src bucket size
    assert BSZ <= 32767

    core = dst // SH
    rel = dst - core * SH
    blk = rel // P
    slot = rel % P
    buck = src // BSZ

    # per (core, block, bucket) edge counts -> chunk counts
    key = (core * NBLK + blk) * NBUCK + buck
    counts = np.bincount(key, minlength=NCORES * NBLK * NBUCK)
    counts = counts.reshape(NCORES, NBLK, NBUCK)
    chunks = -(-counts // P)                                # [NC, NBLK, NBUCK]
    tot = chunks.sum(axis=2)
    # blocks with no edges still need one (dummy) chunk to init PSUM
    empty = tot == 0
    chunks[:, :, 0] = np.where(empty, 1, chunks[:, :, 0])
    tot = chunks.sum(axis=2)

    perm = np.argsort(-tot, axis=1, kind="stable")          # block order per core
    # shared schedule: per (slot-position, bucket) max chunk count over cores
    sorted_chunks = np.take_along_axis(chunks, perm[:, :, None], axis=1)
    NCH4 = sorted_chunks.max(axis=0)                        # [NBLK, NBUCK]

    # global chunk order: batches of GB slots; within a batch buckets are
    # contiguous (one dma_gather per bucket): for b: for q: for s in b: chunks
    GB = GATHER_BLOCKS
    K_of = np.zeros((NBLK, NBUCK), np.int64)                # chunk start of (s, q)
    batches = []   # (s0, s1, gstart, [(q, off_in_batch, nchunks)])
    kg = 0
    for s0 in range(0, NBLK, GB):
        s1 = min(s0 + GB, NBLK)
        gstart = kg
        calls = []
        for q in range(NBUCK):
            off = kg - gstart
            n_q = 0
            for s in range(s0, s1):
                K_of[s, q] = kg
                kg += int(NCH4[s, q])
                n_q += int(NCH4[s, q])
            calls.append((q, off, n_q))
        batches.append((s0, s1, gstart, calls))
    C_total = kg

    inv_perm = np.empty_like(perm)
    np.put_along_axis(
        inv_perm, perm,
        np.broadcast_to(np.arange(NBLK), (NCORES, NBLK)).copy(), axis=1)

    # edge placement: flat position = K_of[s, q]*128 + rank within group
    order = np.argsort(key, kind="stable")
    grp_start = np.zeros(NCORES * NBLK * NBUCK, np.int64)
    grp_start[1:] = np.cumsum(counts.ravel())[:-1]
    pos_in_grp = np.arange(E) - grp_start[key[order]]
    co = core[order]
    s_of = inv_perm[co, blk[order]]
    padpos = K_of[s_of, buck[order]] * P + pos_in_grp

    idx_a = np.zeros((NCORES, C_total * P), np.int16)
    slot_a = np.full((NCORES, C_total * P), 999.0, np.float32)
    w_a = np.zeros((NCORES, C_total * P), np.float32)
    idx_a[co, padpos] = (src[order] - buck[order] * BSZ).astype(np.int16)
    slot_a[co, padpos] = slot[order].astype(np.float32)
    w_a[co, padpos] = w[order]

    # dma_gather index wrap: index i -> [i % 16, i // 16], replicated to 128
    idx16 = idx_a.reshape(NCORES, C_total * 8, 16).transpose(0, 2, 1)
    idx16 = np.ascontiguousarray(
        np.broadcast_to(idx16[:, None, :, :], (NCORES, 8, 16, C_total * 8))
        .reshape(NCORES, P, C_total * 8))
    # per-chunk columns for tensor_scalar scalars
    slot_t = np.ascontiguousarray(
        slot_a.reshape(NCORES, C_total, P).transpose(0, 2, 1))
    w_t = np.ascontiguousarray(w_a.reshape(NCORES, C_total, P).transpose(0, 2, 1))

    deg = np.bincount(dst, minlength=N).astype(np.float32)
    n_core = np.minimum(SH, N - np.arange(NCORES) * SH)
    ids = (np.arange(NCORES)[:, None, None] * SH
           + perm[:, :, None] * P + np.arange(P)[None, None, :])  # [NC, NBLK, P]
    valid = (perm[:, :, None] * P
             + np.arange(P)[None, None, :]) < n_core[:, None, None]
    ids_c = np.where(valid, ids, 0)

    xr = np.zeros((NCORES, NBLK, P, D), np.float32)
    xr[valid] = x[ids_c[valid]]
    xr = xr.reshape(NCORES, SHP, D)

    dt = np.zeros((NCORES, NBLK, P), np.float32)
    dt[valid] = deg[ids_c[valid]]
    deg_t = np.ascontiguousarray(dt.transpose(0, 2, 1))     # [NC, 128, NBLK]

    return dict(
        N=N, D=D, SH=SH, NBLK=NBLK, SHP=SHP, BSZ=BSZ, C_total=C_total,
        NCH4=NCH4, K_of=K_of, batches=batches,
        idx16=idx16, slot_t=slot_t, w_t=w_t,
        xr=xr, deg_t=deg_t, ids=ids, valid=valid,
    )


def _build_program(pre, alpha):
    import concourse.bacc as bacc
    import concourse.bass as bass
    import concourse.tile as tile
    from concourse import mybir

    f32 = mybir.dt.float32
    eq = mybir.AluOpType.is_equal
    mult = mybir.AluOpType.mult
    mx = mybir.AluOpType.max

    N, NBLK, SHP, BSZ = pre["N"], pre["NBLK"], pre["SHP"], pre["BSZ"]
    C_total, NCH4, K_of = pre["C_total"], pre["NCH4"], pre["K_of"]
    batches = pre["batches"]

    nc = bacc.Bacc(None, target_bir_lowering=False)
    x_d = nc.dram_tensor("x", [N, P], f32, kind="ExternalInput")
    idx_d = nc.dram_tensor("idx16", [P, C_total * 8], mybir.dt.int16,
                           kind="ExternalInput")
    slot_d = nc.dram_tensor("slot", [P, C_total], f32, kind="ExternalInput")
    wg_d = nc.dram_tensor("wg", [P, C_total], f32, kind="ExternalInput")
    xr_d = nc.dram_tensor("xr", [SHP, P], f32, kind="ExternalInput")
    deg_d = nc.dram_tensor("deg", [P, NBLK], f32, kind="ExternalInput")
    wt_d = nc.dram_tensor("wt", [P, P], f32, kind="ExternalInput")
    b_d = nc.dram_tensor("b", [1, P], f32, kind="ExternalInput")
    iota_d = nc.dram_tensor("iota", [P, P], f32, kind="ExternalInput")
    y_d = nc.dram_tensor("y", [SHP, P], f32, kind="ExternalOutput")

    with tile.TileContext(nc) as tc:
        with (
            tc.tile_pool(name="const", bufs=1) as cpool,
            tc.tile_pool(name="gather", bufs=2) as gpool,
            tc.tile_pool(name="sw", bufs=6) as swpool,
            tc.tile_pool(name="agg", bufs=3) as aggpool,
            tc.tile_pool(name="xr", bufs=2) as xrpool,
            tc.tile_pool(name="ot", bufs=2) as otpool,
            tc.tile_pool(name="ps1", bufs=2, space="PSUM") as ps1,
            tc.tile_pool(name="ps2", bufs=2, space="PSUM") as ps2,
        ):
            idx_s = cpool.tile([P, C_total * 8], mybir.dt.int16)
            nc.sync.dma_start(out=idx_s[:], in_=idx_d[:, :])
            slot_s = cpool.tile([P, C_total], f32)
            nc.sync.dma_start(out=slot_s[:], in_=slot_d[:, :])
            w_s = cpool.tile([P, C_total], f32)
            nc.sync.dma_start(out=w_s[:], in_=wg_d[:, :])
            wt_s = cpool.tile([P, P], f32)
            nc.sync.dma_start(out=wt_s[:], in_=wt_d[:, :])
            iota_s = cpool.tile([P, P], f32)
            nc.sync.dma_start(out=iota_s[:], in_=iota_d[:, :])

            abb_s = cpool.tile([P, P], f32)
            b_bcast = bass.AP(tensor=b_d, offset=0, ap=[[0, P], [1, P]])
            nc.sync.dma_start(out=abb_s[:], in_=b_bcast)
            nc.scalar.mul(abb_s[:], abb_s[:], float(alpha))

            deg_s = cpool.tile([P, NBLK], f32)
            nc.sync.dma_start(out=deg_s[:], in_=deg_d[:, :])
            invd_s = cpool.tile([P, NBLK], f32)
            nc.vector.tensor_scalar(
                out=invd_s[:], in0=deg_s[:], scalar1=1.0, scalar2=None, op0=mx)
            nc.vector.reciprocal(invd_s[:], invd_s[:])
            nc.scalar.mul(invd_s[:], invd_s[:], float(alpha))

            for (s0, s1, gstart, calls) in batches:
                nb = s1 - s0
                M = sum(n_q for (_, _, n_q) in calls)

                G = gpool.tile([P, M, P], f32)
                for (q, off, n_q) in calls:
                    bsz_q = min(BSZ, N - q * BSZ)
                    for c0 in range(0, n_q, MAX_GATHER_CHUNKS):
                        n_c = min(MAX_GATHER_CHUNKS, n_q - c0)
                        o = off + c0
                        i0 = (gstart + o) * P    # global index position
                        nc.gpsimd.dma_gather(
                            out_ap=G[:, o:o + n_c, :],
                            in_ap=x_d[q * BSZ:q * BSZ + bsz_q, :],
                            idxs_ap=idx_s[:, i0 // 16:(i0 + n_c * P) // 16],
                            num_idxs=n_c * P,
                            num_idxs_reg=n_c * P,
                            elem_size=P,
                        )

                xrt = xrpool.tile([P, nb, P], f32)
                nc.sync.dma_start(
                    out=xrt[:],
                    in_=xr_d[s0 * P:s1 * P, :].rearrange(
                        "(nb p) d -> p nb d", p=P),
                )
                ot = otpool.tile([P, nb, P], f32)

                for s in range(s0, s1):
                    nch = int(NCH4[s].sum())
                    p1 = ps1.tile([P, P], f32)
                    ci = 0
                    for q in range(NBUCK):
                        for c in range(int(NCH4[s, q])):
                            k = int(K_of[s, q]) + c
                            sw = swpool.tile([P, P], f32)
                            nc.vector.tensor_scalar(
                                out=sw[:], in0=iota_s[:],
                                scalar1=slot_s[:, k:k + 1],
                                scalar2=w_s[:, k:k + 1],
                                op0=eq, op1=mult,
                            )
                            nc.tensor.matmul(
                                p1[:], lhsT=G[:, k - gstart, :], rhs=sw[:],
                                start=(ci == 0), stop=(ci == nch - 1),
                            )
                            ci += 1
                    aggT = aggpool.tile([P, P], f32)
                    nc.vector.tensor_copy(aggT[:], p1[:])
                    p2 = ps2.tile([P, P], f32)
                    nc.tensor.matmul(
                        p2[:], lhsT=aggT[:], rhs=wt_s[:], start=True, stop=True)
                    j = s - s0
                    nc.scalar.mul(ot[:, j, :], p2[:], invd_s[:, s:s + 1])
                    nc.vector.tensor_add(ot[:, j, :], ot[:, j, :], abb_s[:])
                    nc.vector.tensor_add(ot[:, j, :], ot[:, j, :], xrt[:, j, :])

                nc.sync.dma_start(
                    out=y_d[s0 * P:s1 * P, :].rearrange(
                        "(nb p) d -> p nb d", p=P),
                    in_=ot[:],
                )

    nc.compile()
    return nc


def kernel(**inputs):
    global LAST_RESULTS
    x = np.ascontiguousarray(np.asarray(inputs["x"], dtype=np.float32))
    ei = np.asarray(inputs["edge_index"])
    w = np.ascontiguousarray(np.asarray(inputs["edge_weight"], dtype=np.float32))
    W = np.asarray(inputs["W"], dtype=np.float32)
    b = np.asarray(inputs["b"], dtype=np.float32)
    alpha = float(np.asarray(inputs["alpha"]))
    src = ei[0].astype(np.int64)
    dst = ei[1].astype(np.int64)

    pre = _preprocess(x, src, dst, w)
    N, D = pre["N"], pre["D"]
    assert D == P

    nc = _build_program(pre, alpha)

    wt = np.ascontiguousarray(W.T)
    iota = np.ascontiguousarray(
        np.broadcast_to(np.arange(P, dtype=np.float32), (P, P)))
    b2 = np.ascontiguousarray(b.reshape(1, P))

    in_maps = []
    for c in range(NCORES):
        in_maps.append({
            "x": x,
            "idx16": pre["idx16"][c],
            "slot": pre["slot_t"][c],
            "wg": pre["w_t"][c],
            "xr": pre["xr"][c],
            "deg": pre["deg_t"][c],
            "wt": wt,
            "b": b2,
            "iota": iota,
        })

    global LAST_NC, LAST_IN_MAPS, LAST_PRE
    LAST_NC, LAST_IN_MAPS, LAST_PRE = nc, in_maps, pre

    from concourse.bass_utils import run_bass_kernel_spmd
    kw = {"trace": True} if TRACE else {}
    res = run_bass_kernel_spmd(
        nc, in_maps, core_ids=list(range(NCORES)), **kw)
    LAST_RESULTS = res

    out = np.empty((N, P), np.float32)
    NBLK = pre["NBLK"]
    valid = pre["valid"]
    ids = pre["ids"]
    for c in range(NCORES):
        y = np.asarray(res.results[c]["y"]).reshape(NBLK, P, P)
        out[ids[c][valid[c]]] = y[valid[c]]
    return out



# revision 27
# speedup vs baseline: 1.1598x; 1.1598x over previous
"""GNN message-passing kernel for Trainium2 (8 NeuronCores, SPMD).

Reference computation:
    msg  = x[src] * edge_weight[:, None]
    agg  = segment_sum(msg, dst, N) / max(segment_sum(1, dst, N), 1)
    out  = x + alpha * (agg @ W.T + b)

Sharding: nodes are sharded across 8 cores by contiguous ranges; edges are
partitioned by dst so scatter-adds stay local; x (bf16) is replicated to
every core's DRAM so the src-row gather is always local.

Per core, dst nodes are grouped into 128-node blocks. A block's incoming
edges are processed in chunks of 128 edges: a dma_gather fetches the 128
src rows in bf16 (256 B each, one per partition), one DVE tensor_scalar
builds the weighted one-hot Sw[e, j] = w'[e]*(slot[e]==j) in bf16 where
w' = w * alpha / max(deg[dst], 1) is folded on the host, and the tensor
engine accumulates aggT[f, n] += Xg[e, f]^T @ Sw[e, n] in PSUM (f32).

Epilogue per block: Activation evacuates aggT to SBUF bf16; the tensor
engine computes p2[f', n] = W.T^T @ aggT + I @ xrbT (the residual
x + alpha*b is pre-added on the host, transposed to [feat, node] layout,
and accumulated into the same PSUM via an identity matmul); Activation
evacuates p2 to the bf16 output tile. All heavy streams (xrbT, y) are in
transposed [128, nodes] layout so DMA descriptors are large and
contiguous.

dma_gather uses int16 indices, so the src space is split into 4 buckets
of <= 32768 rows; each chunk's edges come from a single (block, bucket)
group. Gathers are batched (one dma_gather per bucket per ~12-block
batch) to amortize the ~1us SWDGE per-instruction overhead; the SWDGE
descriptor ring is enlarged to 4096 entries so descriptor generation
overlaps the transfers.

All 8 cores run one shared program: each core orders its blocks by
descending chunk count and the program uses the per-position max, so the
control flow is identical and only the data differs.
"""

import numpy as np

P = 128
NCORES = 8
NBUCK = 4

# set by test harness for profiling; grading leaves these defaults
TRACE = False
LAST_RESULTS = None
GATHER_BLOCKS = 12      # block-slots per gather batch
MAX_GATHER_CHUNKS = 16  # chunks (x128 idx) per dma_gather call (ring cap/2)
DMA_SCRATCH = 65536     # SWDGE ring: 4096 descriptors
TRIM = True             # trim trailing gather descriptors per call


def _to_bf16(a):
    import jax.numpy as jnp
    return np.asarray(jnp.asarray(a, dtype=jnp.bfloat16))


def _preprocess(x, src, dst, w, alpha, b):
    N, D = x.shape
    E = src.shape[0]
    SH = -(-N // NCORES)          # nodes per core shard
    NBLK = -(-SH // P)            # 128-node blocks per core
    SHP = NBLK * P                # padded shard size

    deg = np.bincount(dst, minlength=N).astype(np.float32)
    wp = (w * alpha / np.maximum(deg, 1.0)[dst]).astype(np.float32)

    core = dst // SH
    rel = dst - core * SH
    blk = rel // P
    slot = rel % P

    # bucket boundaries: scan the (a, a, a, N-3a) family for the split that
    # minimizes total shared-schedule chunks (each bucket <= 32767 rows)
    cb = (core * NBLK + blk)
    best = None
    for a in range(25000, 32768, 512):
        sizes = np.array([a, a, a, N - 3 * a])
        if sizes[3] <= 0 or sizes.max() > 32767:
            continue
        bounds = np.concatenate([[0], np.cumsum(sizes)])
        bk = np.searchsorted(bounds, src, side="right") - 1
        cnt = np.bincount(cb * NBUCK + bk, minlength=NCORES * NBLK * NBUCK)
        ch = -(-cnt.reshape(NCORES, NBLK, NBUCK) // P)
        t = ch.sum(axis=2)
        pm = np.argsort(-t, axis=1, kind="stable")
        sc = np.take_along_axis(ch, pm[:, :, None], axis=1)
        total = sc.max(axis=0).sum()
        if best is None or total < best[0]:
            best = (total, bounds)
    bounds = best[1]
    bsizes = np.diff(bounds)
    buck = (np.searchsorted(bounds, src, side="right") - 1).astype(np.int64)

    # per (core, block, bucket) edge counts -> chunk counts
    key = (core * NBLK + blk) * NBUCK + buck
    counts = np.bincount(key, minlength=NCORES * NBLK * NBUCK)
    counts = counts.reshape(NCORES, NBLK, NBUCK)
    chunks = -(-counts // P)                                # [NC, NBLK, NBUCK]
    tot = chunks.sum(axis=2)
    # blocks with no edges still need one (dummy) chunk to init PSUM
    empty = tot == 0
    chunks[:, :, 0] = np.where(empty, 1, chunks[:, :, 0])
    tot = chunks.sum(axis=2)

    perm = np.argsort(-tot, axis=1, kind="stable")          # block order per core
    # shared schedule: per (slot-position, bucket) max chunk count over cores
    sorted_chunks = np.take_along_axis(chunks, perm[:, :, None], axis=1)
    NCH4 = sorted_chunks.max(axis=0)                        # [NBLK, NBUCK]
    # per-position max raw edge count (for trailing-descriptor trim)
    sorted_counts = np.take_along_axis(counts, perm[:, :, None], axis=1)
    MC4 = sorted_counts.max(axis=0)                         # [NBLK, NBUCK]

    # global chunk order: batches of GB block-slots. Per (batch, bucket),
    # groups are packed into gather calls of <= MAX_GATHER_CHUNKS chunks,
    # split at group boundaries, with the call's max-waste group last so the
    # trailing-descriptor trim recovers the most padding.
    GB = GATHER_BLOCKS
    MAXC = MAX_GATHER_CHUNKS

    def a16(v):
        return max(16, -(-v // 16) * 16)

    K_of = np.zeros((NBLK, NBUCK), np.int64)                # chunk start of (s, q)
    batches = []   # (s0, s1, gstart, M, [(q, off, ncols, nidx_full, nidx_trim)])
    kg = 0
    for s0 in range(0, NBLK, GB):
        s1 = min(s0 + GB, NBLK)
        gstart = kg
        calls = []
        for q in range(NBUCK):
            groups = [(s, int(NCH4[s, q]), int(MC4[s, q]))
                      for s in range(s0, s1) if NCH4[s, q] > 0]
            groups.sort(key=lambda g: g[1] * P - a16(g[2]))
            parts, cur, cur_ch = [], [], 0
            for g in groups:
                if cur and cur_ch + g[1] > MAXC:
                    parts.append(cur)
                    cur, cur_ch = [], 0
                cur.append(g)
                cur_ch += g[1]
            if cur:
                parts.append(cur)
            for part in parts:
                mi = max(range(len(part)),
                         key=lambda i: part[i][1] * P - a16(part[i][2]))
                part.append(part.pop(mi))
                off = kg - gstart
                ncols = sum(g[1] for g in part)
                for g in part:
                    K_of[g[0], q] = kg
                    kg += g[1]
                tail = part[-1]
                full = ncols * P
                trimmed = full - (tail[1] * P - a16(tail[2]))
                calls.append((q, off, ncols, full, trimmed))
        batches.append((s0, s1, gstart, kg - gstart, calls))
    C_total = kg
    Mmax = max(M for (_, _, _, M, _) in batches)
    # filler gathers so batches 0/1 initialize every G buffer column with
    # finite data (later trimmed calls leave stale-but-finite bytes)
    batches = [
        (s0, s1, gstart, M,
         calls + ([(0, M, Mmax - M, (Mmax - M) * P, (Mmax - M) * P)]
                  if bi < 2 and M < Mmax and bi + 1 < len(batches) else []))
        for bi, (s0, s1, gstart, M, calls) in enumerate(batches)
    ]

    inv_perm = np.empty_like(perm)
    np.put_along_axis(
        inv_perm, perm,
        np.broadcast_to(np.arange(NBLK), (NCORES, NBLK)).copy(), axis=1)

    # edge placement: flat position = K_of[s, q]*128 + rank within group
    order = np.argsort(key, kind="stable")
    grp_start = np.zeros(NCORES * NBLK * NBUCK, np.int64)
    grp_start[1:] = np.cumsum(counts.ravel())[:-1]
    pos_in_grp = np.arange(E) - grp_start[key[order]]
    co = core[order]
    s_of = inv_perm[co, blk[order]]
    padpos = K_of[s_of, buck[order]] * P + pos_in_grp

    idx_a = np.zeros((NCORES, C_total * P), np.int16)
    slot_a = np.full((NCORES, C_total * P), 999.0, np.float32)
    w_a = np.zeros((NCORES, C_total * P), np.float32)
    idx_a[co, padpos] = (src[order] - bounds[buck[order]]).astype(np.int16)
    slot_a[co, padpos] = slot[order].astype(np.float32)
    w_a[co, padpos] = wp[order]

    # dma_gather index wrap: index i -> [i % 16, i // 16], replicated to 128
    idx16 = idx_a.reshape(NCORES, C_total * 8, 16).transpose(0, 2, 1)
    idx16 = np.ascontiguousarray(
        np.broadcast_to(idx16[:, None, :, :], (NCORES, 8, 16, C_total * 8))
        .reshape(NCORES, P, C_total * 8))
    # per-chunk columns for tensor_scalar scalars (must be f32 when op0 is
    # is_equal)
    slot_t = np.ascontiguousarray(
        slot_a.reshape(NCORES, C_total, P).transpose(0, 2, 1))
    w_t = np.ascontiguousarray(
        w_a.reshape(NCORES, C_total, P).transpose(0, 2, 1))

    n_core = np.minimum(SH, N - np.arange(NCORES) * SH)
    ids = (np.arange(NCORES)[:, None, None] * SH
           + perm[:, :, None] * P + np.arange(P)[None, None, :])  # [NC, NBLK, P]
    valid = (perm[:, :, None] * P
             + np.arange(P)[None, None, :]) < n_core[:, None, None]
    ids_c = np.where(valid, ids, 0)

    # residual + bias, transposed to [feat, node]: xrbT[c, f, s*128+p]
    xrb = np.zeros((NCORES, NBLK, P, D), np.float32)
    xrb[valid] = x[ids_c[valid]] + alpha * b[None, :]
    xrbT = _to_bf16(
        np.ascontiguousarray(
            xrb.reshape(NCORES, SHP, D).transpose(0, 2, 1)))  # [NC, 128, SHP]

    return dict(
        N=N, D=D, SH=SH, NBLK=NBLK, SHP=SHP, C_total=C_total,
        NCH4=NCH4, MC4=MC4, K_of=K_of, batches=batches, bounds=bounds,
        idx16=idx16, slot_t=slot_t, w_t=w_t,
        xrbT=xrbT, ids=ids, valid=valid,
    )


def _build_program(pre, alpha=None):
    import concourse.bacc as bacc
    import concourse.tile as tile
    from concourse import mybir

    f32 = mybir.dt.float32
    bf16 = mybir.dt.bfloat16
    eq = mybir.AluOpType.is_equal
    mult = mybir.AluOpType.mult

    N, NBLK, SHP = pre["N"], pre["NBLK"], pre["SHP"]
    C_total, NCH4, K_of = pre["C_total"], pre["NCH4"], pre["K_of"]
    MC4, bounds = pre["MC4"], pre["bounds"]
    batches = pre["batches"]
    Mmax = max(M for (_, _, _, M, _) in batches)
    Mcap = max(M + sum(n for (_, o, n, _, _) in calls if o >= M)
               for (_, _, _, M, calls) in batches)   # incl. filler columns

    nc = bacc.Bacc(None, target_bir_lowering=False,
                   dynamic_dma_scratch_size=DMA_SCRATCH)
    xw_d = nc.dram_tensor("xw", [N, P], bf16, kind="ExternalInput")
    idx_d = nc.dram_tensor("idx16", [P, C_total * 8], mybir.dt.int16,
                           kind="ExternalInput")
    slot_d = nc.dram_tensor("slot", [P, C_total], f32, kind="ExternalInput")
    wg_d = nc.dram_tensor("wg", [P, C_total], f32, kind="ExternalInput")
    xrbt_d = nc.dram_tensor("xrbt", [P, SHP], bf16, kind="ExternalInput")
    wt_d = nc.dram_tensor("wt", [P, P], bf16, kind="ExternalInput")
    iota_d = nc.dram_tensor("iota", [P, P], bf16, kind="ExternalInput")
    id_d = nc.dram_tensor("ident", [P, P], bf16, kind="ExternalInput")
    y_d = nc.dram_tensor("y", [P, SHP], bf16, kind="ExternalOutput")

    with tile.TileContext(nc) as tc:
        with (
            tc.tile_pool(name="const", bufs=1) as cpool,
            tc.tile_pool(name="sw", bufs=6) as swpool,
            tc.tile_pool(name="ix", bufs=2) as ixpool,
            tc.tile_pool(name="agg", bufs=3) as aggpool,
            tc.tile_pool(name="xrb", bufs=2) as xrpool,
            tc.tile_pool(name="ot", bufs=2) as otpool,
            tc.tile_pool(name="ps1", bufs=4, space="PSUM") as ps1,
            tc.tile_pool(name="ps2", bufs=2, space="PSUM") as ps2,
        ):
            slot_s = cpool.tile([P, C_total], f32)
            nc.sync.dma_start(out=slot_s[:], in_=slot_d[:, :])
            w_s = cpool.tile([P, C_total], f32)
            nc.sync.dma_start(out=w_s[:], in_=wg_d[:, :])
            wt_s = cpool.tile([P, P], bf16)
            nc.sync.dma_start(out=wt_s[:], in_=wt_d[:, :])
            iota_s = cpool.tile([P, P], bf16)
            nc.sync.dma_start(out=iota_s[:], in_=iota_d[:, :])
            id_s = cpool.tile([P, P], bf16)
            nc.sync.dma_start(out=id_s[:], in_=id_d[:, :])

            # manually double-buffered gather destinations
            G2 = [cpool.tile([P, Mcap, P], bf16, name=f"Gbuf{i}")
                  for i in range(2)]

            for bi, (s0, s1, gstart, M, calls) in enumerate(batches):
                nb = s1 - s0
                G = G2[bi % 2]

                # this batch's slice of the gather indices (the filler call
                # of batches 0/1 reads past M into the next batch's region)
                mi = min(Mcap, C_total - gstart)
                ix = ixpool.tile([P, Mcap * 8], mybir.dt.int16)
                nc.sync.dma_start(
                    out=ix[:, :mi * 8],
                    in_=idx_d[:, gstart * 8:(gstart + mi) * 8])

                for (q, off, ncols, nfull, ntrim) in calls:
                    n_idx = ntrim if (TRIM and bi >= 2) else nfull
                    nc.gpsimd.dma_gather(
                        out_ap=G[:, off:off + ncols, :],
                        in_ap=xw_d[int(bounds[q]):int(bounds[q + 1]), :],
                        idxs_ap=ix[:, off * 8:off * 8 + (n_idx + 15) // 16],
                        num_idxs=n_idx,
                        num_idxs_reg=n_idx,
                        elem_size=P,
                    )

                xrt = xrpool.tile([P, nb * P], bf16)
                nc.sync.dma_start(
                    out=xrt[:], in_=xrbt_d[:, s0 * P:s1 * P])
                ot = otpool.tile([P, nb * P], bf16)

                for s in range(s0, s1):
                    nch = int(NCH4[s].sum())
                    p1 = ps1.tile([P, P], f32)
                    ci = 0
                    for q in range(NBUCK):
                        for c in range(int(NCH4[s, q])):
                            k = int(K_of[s, q]) + c
                            sw = swpool.tile([P, P], bf16)
                            nc.vector.tensor_scalar(
                                out=sw[:], in0=iota_s[:],
                                scalar1=slot_s[:, k:k + 1],
                                scalar2=w_s[:, k:k + 1],
                                op0=eq, op1=mult,
                            )
                            nc.tensor.matmul(
                                p1[:], lhsT=G[:, k - gstart, :], rhs=sw[:],
                                start=(ci == 0), stop=(ci == nch - 1),
                            )
                            ci += 1
                    aggT = aggpool.tile([P, P], bf16)
                    nc.scalar.copy(aggT[:], p1[:])
                    j = s - s0
                    p2 = ps2.tile([P, P], f32)
                    nc.tensor.matmul(
                        p2[:], lhsT=wt_s[:], rhs=aggT[:],
                        start=True, stop=False)
                    nc.tensor.matmul(
                        p2[:], lhsT=id_s[:], rhs=xrt[:, j * P:(j + 1) * P],
                        start=False, stop=True)
                    nc.scalar.copy(ot[:, j * P:(j + 1) * P], p2[:])

                nc.sync.dma_start(
                    out=y_d[:, s0 * P:s1 * P], in_=ot[:])

    nc.compile()
    return nc


def kernel(**inputs):
    global LAST_RESULTS
    x = np.ascontiguousarray(np.asarray(inputs["x"], dtype=np.float32))
    ei = np.asarray(inputs["edge_index"])
    w = np.ascontiguousarray(np.asarray(inputs["edge_weight"], dtype=np.float32))
    W = np.asarray(inputs["W"], dtype=np.float32)
    b = np.asarray(inputs["b"], dtype=np.float32)
    alpha = float(np.asarray(inputs["alpha"]))
    src = ei[0].astype(np.int64)
    dst = ei[1].astype(np.int64)

    pre = _preprocess(x, src, dst, w, alpha, b)
    N, D = pre["N"], pre["D"]
    assert D == P

    nc = _build_program(pre)

    xw = _to_bf16(x)
    wt = _to_bf16(np.ascontiguousarray(W.T))
    iota = _to_bf16(np.broadcast_to(
        np.arange(P, dtype=np.float32), (P, P)).copy())
    ident = _to_bf16(np.eye(P, dtype=np.float32))

    in_maps = []
    for c in range(NCORES):
        in_maps.append({
            "xw": xw,
            "idx16": pre["idx16"][c],
            "slot": pre["slot_t"][c],
            "wg": pre["w_t"][c],
            "xrbt": pre["xrbT"][c],
            "wt": wt,
            "iota": iota,
            "ident": ident,
        })

    global LAST_NC, LAST_IN_MAPS, LAST_PRE
    LAST_NC, LAST_IN_MAPS, LAST_PRE = nc, in_maps, pre

    from concourse.bass_utils import run_bass_kernel_spmd
    kw = {"trace": True} if TRACE else {}
    res = run_bass_kernel_spmd(
        nc, in_maps, core_ids=list(range(NCORES)), **kw)
    LAST_RESULTS = res

    out = np.empty((N, P), np.float32)
    NBLK = pre["NBLK"]
    valid = pre["valid"]
    ids = pre["ids"]
    for c in range(NCORES):
        yT = np.asarray(res.results[c]["y"]).astype(np.float32)  # [128, SHP]
        y = np.ascontiguousarray(yT.T).reshape(NBLK, P, P)
        out[ids[c][valid[c]]] = y[valid[c]]
    return out
